# revision 17
# baseline (speedup 1.0000x reference)
"""Trainium2 Bass kernel for nn_Cross_Attention (8-core data-parallel over batch).

Reference computation per batch item (one NeuronCore each):
  kvf  = conv1x1(kv, qkv1_w)                    # [384, H, W]
  kvd  = depthwise3x3(kvf, qkv2_w, pad=1)       # [384, H, W]
  k, v = split(kvd); q/k L2-normalized over hw per channel row
  attn = softmax(scale * qn @ kn^T)             # block-diag [192, 192]
  out  = proj1x1(attn @ v, proj_w)              # [192, H, W]

Fully-streamed design: one pass over 512-pixel stripes runs conv1,
depthwise (diagonal matmuls with contiguous 2D APs into column-padded
halo ring slots), DMA-transposes of k/q chunks, and Gram accumulation
with DEFERRED normalization (1/||q||, 1/||k|| and `scale` are applied
to the Gram matrix afterwards), so the PE never hits a phase barrier.
The projection is folded into the attention matrix (out = (proj@A)@v)
and v stays resident in SBUF.
"""

import sys

sys.path.insert(0, "/opt/trn_rl_repo")

import numpy as np

import concourse.bass as bass
import concourse.tile as tile
from concourse import bacc, mybir
from concourse.bass_utils import run_bass_kernel_spmd
from concourse.bass_interp import get_hw_module

F32 = mybir.dt.float32
F16 = mybir.dt.float16

C = 192          # channels
C2 = 384         # conv1 output channels
HEADS = 8
CD = C // HEADS
W = 128
H = 128
HWTOT = H * W
PT = 512                 # pixels per stripe (4 image rows)
RPT = PT // W            # rows per stripe = 4
NT = HWTOT // PT         # 32 stripes
CHS = 4                  # stripes per transpose/load chunk
CPX = CHS * PT           # 2048 px per chunk
NCH = NT // CHS          # 8 chunks
EPS = 1e-12

# taps: weight index wi = (dr+1)*3 + (dc+1)
TAPS = [(dr, dc) for dr in (-1, 0, 1) for dc in (-1, 0, 1)]


def sl(i, size=PT):
    return slice(i * size, (i + 1) * size)


def emit_kernel(tc, io):
    nc = tc.nc
    kv, q = io["kv"], io["q"]
    w1t, w2d, wpt = io["w1t"], io["w2d"], io["wpt"]
    w2col_d = io["w2col"]
    w2dn = io["w2dn"]
    ident, mask01, scale192 = io["ident"], io["mask01"], io["scale192"]
    out = io["out"]

    from contextlib import ExitStack
    st = ExitStack()
    wp = st.enter_context(tc.tile_pool(name="weights", bufs=1))
    sml = st.enter_context(tc.tile_pool(name="small", bufs=1))
    big = st.enter_context(tc.tile_pool(name="big", bufs=1))
    kvfp = st.enter_context(tc.tile_pool(name="kvf", bufs=1))
    chp = st.enter_context(tc.tile_pool(name="chunks", bufs=1))

    # ---- weights / constants to SBUF ----
    w1ta = wp.tile([128, C2], F16); nc.sync.dma_start(w1ta[:], w1t[0:128, :])
    w1tb = wp.tile([64, C2], F16); nc.sync.dma_start(w1tb[:], w1t[128:C, :])
    wpta = wp.tile([128, C], F16); nc.sync.dma_start(wpta[:], wpt[0:128, :])
    wptb = wp.tile([64, C], F16); nc.sync.dma_start(wptb[:], wpt[128:C, :])
    id16 = wp.tile([128, 128], F16); nc.sync.dma_start(id16[:], ident[:])
    mka = wp.tile([128, C], F16); nc.sync.dma_start(mka[:], mask01[0:128, :])
    mkb = wp.tile([64, C], F16); nc.sync.dma_start(mkb[:], mask01[128:C, :])
    sca = wp.tile([128, 1], F32); nc.sync.dma_start(sca[:], scale192[0:128, :])
    scb = wp.tile([64, 1], F32); nc.sync.dma_start(scb[:], scale192[128:C, :])
    w2sb = wp.tile([128, 27, 128], F16)
    nc.sync.dma_start(w2sb[:], w2d.rearrange("t p c -> p t c"))
    w2cs = wp.tile([128, 27], F32)
    nc.sync.dma_start(w2cs[:], w2col_d[:])
    w2ns = wp.tile([128, 18, 128], F16)
    nc.sync.dma_start(w2ns[:], w2dn.rearrange("t p c -> p t c"))

    # ---- persistent big tensors ----
    # v pieces in PSUM-native partition layout:
    #   vAB rows 0:64  = v[64:128] (dw mc2 psum rows 0:64)
    #   vAB rows 64:128 = v[0:64]  (dw mc1 psum rows 64:128)
    #   vC  rows 64:128 = v[128:192] (dw mc2 psum rows 64:128); rows 0:64 unused
    vAB = big.tile([128, HWTOT], F16, name="vAB")
    vC = big.tile([128, HWTOT], F16, name="vC")

    # norm part-sums (one column per chunk)
    kq2a = sml.tile([128, NCH], F32)
    kq2b = sml.tile([64, NCH], F32)
    qq2a = sml.tile([128, NCH], F32)
    qq2b = sml.tile([64, NCH], F32)

    # kvf halo ring: per mc, 4 FLAT slots [128, 896]: 8-el front pad,
    # 6 image rows (4s-1..4s+4) at [8+128r : 8+128r+128], zero tail pad.
    # Flat layout lets every depthwise tap stream a contiguous 512 slice;
    # the 8 wrapped edge columns are fixed by tiny N=4 correction matmuls.
    kvf = [[kvfp.tile([128, 896], F16, tag=f"kvf{mc}_{j}",
                      name=f"kvf{mc}_{j}") for j in range(4)]
           for mc in range(3)]
    for mc in range(3):
        for j in range(4):
            nc.vector.memset(kvf[mc][j][:], 0.0)

    # chunk rings (double buffered)
    kv16a = [chp.tile([128, CPX], F16, name=f"kv16a{i}", tag=f"kva{i}") for i in range(2)]
    kv16b = [chp.tile([64, CPX], F16, name=f"kv16b{i}", tag=f"kvb{i}") for i in range(2)]
    q16a = [chp.tile([128, CPX], F16, name=f"q16a{i}", tag=f"qa{i}") for i in range(2)]
    q16b = [chp.tile([64, CPX], F16, name=f"q16b{i}", tag=f"qb{i}") for i in range(2)]
    k16a = [chp.tile([128, CPX], F16, name=f"k16a{i}", tag=f"ka{i}") for i in range(2)]
    k16b = [chp.tile([64, CPX], F16, name=f"k16b{i}", tag=f"kb{i}") for i in range(2)]
    qB = [chp.tile([128, CHS * RPT, C], F16, name=f"qB{i}", tag=f"qB{i}") for i in range(2)]
    kB = [chp.tile([128, CHS * RPT, C], F16, name=f"kB{i}", tag=f"kB{i}") for i in range(2)]
    sqt = chp.tile([128, CPX], F16, tag="sqt")  # square scratch
    accp = st.enter_context(tc.tile_pool(name="accp", bufs=2))

    def load_chunk(ch, granular=False):
        cb = ch % 2
        c0 = ch * CPX
        if granular:
            # per-stripe pieces so the first conv can start sooner
            for j in range(CHS):
                p0 = c0 + j * PT
                nc.gpsimd.dma_start(kv16a[cb][:, sl(j)], kv[0:128, p0:p0 + PT])
                nc.gpsimd.dma_start(kv16b[cb][:, sl(j)], kv[128:C, p0:p0 + PT])
        else:
            nc.gpsimd.dma_start(kv16a[cb][:], kv[0:128, c0:c0 + CPX])
            nc.gpsimd.dma_start(kv16b[cb][:], kv[128:C, c0:c0 + CPX])
        nc.gpsimd.dma_start(q16a[cb][:], q[0:128, c0:c0 + CPX])
        nc.gpsimd.dma_start(q16b[cb][:], q[128:C, c0:c0 + CPX])

    MUL0 = mybir.AluOpType.mult

    def q_prep(ch):
        cb = ch % 2
        nc.vector.scalar_tensor_tensor(out=sqt[:], in0=q16a[cb][:],
                                       scalar=1.0, in1=q16a[cb][:],
                                       op0=MUL0, op1=MUL0,
                                       accum_out=qq2a[:, ch:ch + 1])
        nc.vector.scalar_tensor_tensor(out=sqt[0:64, :], in0=q16b[cb][:],
                                       scalar=1.0, in1=q16b[cb][:],
                                       op0=MUL0, op1=MUL0,
                                       accum_out=qq2b[:, ch:ch + 1])
        nc.sync.dma_start_transpose(qB[cb][:, :, 0:128], q16a[cb][:])
        nc.sync.dma_start_transpose(qB[cb][:, :, 128:C], q16b[cb][:])

    def k_prep(ch):
        cb = ch % 2
        nc.scalar.activation(sqt[:], k16a[cb][:],
                             mybir.ActivationFunctionType.Square,
                             accum_out=kq2a[:, ch:ch + 1])
        nc.scalar.activation(sqt[0:64, :], k16b[cb][:],
                             mybir.ActivationFunctionType.Square,
                             accum_out=kq2b[:, ch:ch + 1])
        nc.sync.dma_start_transpose(kB[cb][:, :, 0:128], k16a[cb][:])
        nc.sync.dma_start_transpose(kB[cb][:, :, 128:C], k16b[cb][:])

    load_chunk(0)

    with tc.tile_pool(name="psG", bufs=1, space="PSUM") as psG:
        G0 = psG.tile([128, C], F32, name="G0")   # G^T rows d=0:128, cols c
        G1 = psG.tile([64, C], F32, name="G1")    # G^T rows d=128:192

        # ================= streamed phase B =================
        with tc.tile_pool(name="psC", bufs=3, space="PSUM") as psConv, \
             tc.tile_pool(name="psD", bufs=2, space="PSUM") as psDW:
            # PE warm-up: ~5us of junk matmuls while the first casts land,
            # so HAM reaches K=8/8 before real work and the PE isn't idle.
            wps = psConv.tile([128, PT], F32, tag="cv", name="wps")
            wsrc = w2sb[:].rearrange("p t c -> p (t c)")
            for wu in range(24):
                nc.tensor.matmul(wps[:], id16[:], wsrc[:, 0:PT],
                                 start=True, stop=True)

            def conv_stripe(s):
                ch, j = s // CHS, s % CHS
                cb = ch % 2
                for mc in range(3):
                    ps = psConv.tile([128, PT], F32, tag="cv")
                    nc.tensor.matmul(ps[:], w1ta[:, mc * 128:(mc + 1) * 128],
                                     kv16a[cb][:, sl(j)], start=True,
                                     stop=False)
                    nc.tensor.matmul(ps[:], w1tb[:, mc * 128:(mc + 1) * 128],
                                     kv16b[cb][:, sl(j)], start=False,
                                     stop=True)
                    nc.any.tensor_copy(kvf[mc][s % 4][:, 136:648], ps[:])
                    if s > 0:
                        nc.vector.tensor_copy(
                            kvf[mc][(s - 1) % 4][:, 648:776], ps[:, 0:W])
                    if s < NT - 1:
                        nc.vector.tensor_copy(
                            kvf[mc][(s + 1) % 4][:, 8:136], ps[:, PT - W:PT])

            NTAPS = [(dr, dc) for dr in (-1, 0, 1) for dc in (-1, 1)]

            def dw_stripe(s):
                ch, j = s // CHS, s % CHS
                cb = ch % 2
                slot = s % 4
                for mc in range(3):
                    sv = kvf[mc][slot]
                    f3 = sv[:].rearrange("p (r c) -> p r c", c=W)
                    pd = psDW.tile([128, PT], F32, tag="dw")
                    pd3 = pd[:].rearrange("p (r c) -> p r c", c=W)
                    for ti, (dr, dc) in enumerate(TAPS):
                        wi = (dr + 1) * 3 + (dc + 1)
                        st0 = 8 + (1 + dr) * W + dc
                        nc.tensor.matmul(
                            pd[:], w2sb[:, mc * 9 + wi, :],
                            sv[:, st0:st0 + PT],
                            start=(ti == 0), stop=False)
                    # wrap corrections: negated weights on the 8 edge cols
                    for ki, (dr, dc) in enumerate(NTAPS):
                        if dc == -1:
                            mv = f3[:, 1 + dr:5 + dr, 7:8]
                            ov = pd3[:, :, 0:1]
                        else:
                            mv = f3[:, 2 + dr:6 + dr, 8:9]
                            ov = pd3[:, :, 127:128]
                        nc.tensor.matmul(
                            ov, w2ns[:, mc * 6 + ki, :], mv,
                            start=False, stop=(ki == len(NTAPS) - 1))
                    if mc == 0:
                        nc.any.tensor_copy(k16a[cb][:, sl(j)], pd[:])
                    elif mc == 1:
                        nc.any.tensor_copy(k16b[cb][:, sl(j)], pd[0:64, :])
                        nc.any.tensor_copy(vAB[64:128, sl(s)], pd[64:128, :])
                    else:
                        nc.any.tensor_copy(vAB[0:64, sl(s)], pd[0:64, :])
                        nc.any.tensor_copy(vC[64:128, sl(s)], pd[64:128, :])

            def gram_chunk(ch):
                cb = ch % 2
                for blk in range(CHS * RPT):
                    s0 = (ch == 0 and blk == 0)
                    s1 = (ch == NCH - 1 and blk == CHS * RPT - 1)
                    nc.tensor.matmul(G0[:], kB[cb][:, blk, 0:128],
                                     qB[cb][:, blk, :], start=s0, stop=s1)
                    nc.tensor.matmul(G1[:], kB[cb][:, blk, 128:C],
                                     qB[cb][:, blk, :], start=s0, stop=s1)

            for ch in range(NCH):
                if ch + 1 < NCH:
                    load_chunk(ch + 1)
                q_prep(ch)
                for j in range(CHS):
                    s = ch * CHS + j
                    conv_stripe(s)
                    if s == NT - 1:
                        # zero the never-written edge halo row (image row 128)
                        for mc in range(3):
                            nc.vector.memset(kvf[mc][s % 4][:, 648:776], 0.0)
                        dw_stripe(s - 1)
                        dw_stripe(s)
                        k_prep(NCH - 1)
                    elif s > 0:
                        dw_stripe(s - 1)
                        if (s - 1) % CHS == CHS - 1:
                            k_prep((s - 1) // CHS)
                    if j == 1 and ch > 0:
                        gram_chunk(ch - 1)
            gram_chunk(NCH - 1)

        # ============ phase C: norms, softmax, fused proj ============
        nk2a = sml.tile([128, 1], F32)
        nk2b = sml.tile([64, 1], F32)
        nq2a = sml.tile([128, 1], F32)
        nq2b = sml.tile([64, 1], F32)
        spa = sml.tile([128, 1], F32)
        spb = sml.tile([64, 1], F32)
        for dst, src in ((nk2a, kq2a), (nk2b, kq2b),
                         (nq2a, qq2a), (nq2b, qq2b)):
            nc.vector.reduce_sum(dst[:], src[:], axis=mybir.AxisListType.X)
            nc.scalar.sqrt(dst[:], dst[:])
            nc.vector.tensor_scalar_max(dst[:], dst[:], EPS)
            nc.vector.reciprocal(dst[:], dst[:])
        # spa = scale / ||q||
        nc.vector.tensor_tensor(out=spa[:], in0=nq2a[:], in1=sca[:],
                                op=mybir.AluOpType.mult)
        nc.vector.tensor_tensor(out=spb[:], in0=nq2b[:], in1=scb[:],
                                op=mybir.AluOpType.mult)

        # Gt16 = G^T * (1/||k_d||) (per-partition), f16
        Gt16a = sml.tile([128, C], F16)
        Gt16b = sml.tile([64, C], F16)
        nc.vector.tensor_scalar_mul(Gt16a[:], G0[:], nk2a[:])
        nc.vector.tensor_scalar_mul(Gt16b[:], G1[:], nk2b[:])

    # psG closed; transpose S = Gt^T back to [c, d] layout
    E16a = sml.tile([128, C], F16)
    E16b = sml.tile([64, C], F16)
    dena = sml.tile([128, 1], F32)
    denb = sml.tile([64, 1], F32)
    with tc.tile_pool(name="psS", bufs=1, space="PSUM") as psS:
        S1 = psS.tile([128, C], F16, name="S1")   # rows c=0:128, cols d
        S2 = psS.tile([64, C], F16, name="S2")    # rows c=128:192
        nc.tensor.transpose(S1[:, 0:128], Gt16a[:, 0:128], id16[:])
        nc.tensor.transpose(S1[:, 128:C], Gt16b[:, 0:128], id16[0:64, 0:64])
        nc.tensor.transpose(S2[:, 0:128], Gt16a[:, 128:C], id16[:])
        nc.tensor.transpose(S2[:, 128:C], Gt16b[:, 128:C], id16[0:64, 0:64])

        # E = exp(spa * S) (per-partition scale), then mask + row-sum
        nc.scalar.activation(E16a[:], S1[:], mybir.ActivationFunctionType.Exp,
                             scale=spa[:])
        nc.scalar.activation(E16b[:], S2[:], mybir.ActivationFunctionType.Exp,
                             scale=spb[:])
    nc.vector.scalar_tensor_tensor(out=E16a[:], in0=E16a[:], scalar=1.0,
                                   in1=mka[:], op0=mybir.AluOpType.mult,
                                   op1=mybir.AluOpType.mult,
                                   accum_out=dena[:])
    nc.vector.scalar_tensor_tensor(out=E16b[:], in0=E16b[:], scalar=1.0,
                                   in1=mkb[:], op0=mybir.AluOpType.mult,
                                   op1=mybir.AluOpType.mult,
                                   accum_out=denb[:])
    nc.vector.reciprocal(dena[:], dena[:])
    nc.vector.reciprocal(denb[:], denb[:])
    # wpt' = wpt / den (per-partition row scale of proj^T)
    wp2a = sml.tile([128, C], F16)
    wp2b = sml.tile([64, C], F16)
    nc.vector.tensor_scalar_mul(wp2a[:], wpta[:], dena[:])
    nc.vector.tensor_scalar_mul(wp2b[:], wptb[:], denb[:])

    # MT[d, o] = sum_c E16[c, d] * wpt'[c, o], rows permuted to match v:
    #   MT1 rows 0:64 = d 64:128, rows 64:128 = d 0:64  (matches vAB)
    #   MT2 rows 64:128 = d 128:192                     (matches vC)
    MTab = sml.tile([128, C], F16)
    MTc = sml.tile([128, C], F16)
    mtt0 = sml.tile([64, C], F16)
    mtt128 = sml.tile([64, C], F16)
    with tc.tile_pool(name="psM", bufs=1, space="PSUM") as psM:
        MTd64 = psM.tile([64, C], F32, name="MTd64")
        MTd0 = psM.tile([64, C], F32, name="MTd0")
        MTd128 = psM.tile([64, C], F32, name="MTd128")
        for MTx, lo, hi in ((MTd64, 64, 128), (MTd0, 0, 64),
                            (MTd128, 128, C)):
            nc.tensor.matmul(MTx[:], E16a[:, lo:hi], wp2a[:],
                             start=True, stop=False)
            nc.tensor.matmul(MTx[:], E16b[:, lo:hi], wp2b[:],
                             start=False, stop=True)
        nc.any.tensor_copy(MTab[0:64, :], MTd64[:])
        nc.any.tensor_copy(mtt0[:], MTd0[:])
        nc.any.tensor_copy(mtt128[:], MTd128[:])
        # cross-partition placement via SBUF->SBUF DMA
        nc.sync.dma_start(MTab[64:128, :], mtt0[:])
        nc.sync.dma_start(MTc[64:128, :], mtt128[:])

    # ================= phase D: out = MT^T @ v =================
    with tc.tile_pool(name="ost", bufs=3) as ost, \
         tc.tile_pool(name="psO", bufs=2, space="PSUM") as psO:
        for s in range(NT):
            Oa = psO.tile([128, PT], F32, tag="Oa")
            Ob = psO.tile([64, PT], F32, tag="Ob")
            nc.tensor.matmul(Oa[:], MTab[:, 0:128], vAB[:, sl(s)],
                             start=True, stop=False)
            nc.tensor.matmul(Oa[:], MTc[64:128, 0:128], vC[64:128, sl(s)],
                             start=False, stop=True)
            nc.tensor.matmul(Ob[:], MTab[:, 128:C], vAB[:, sl(s)],
                             start=True, stop=False)
            nc.tensor.matmul(Ob[:], MTc[64:128, 128:C], vC[64:128, sl(s)],
                             start=False, stop=True)
            fa = ost.tile([128, PT], F32, tag="fa")
            fb = ost.tile([64, PT], F32, tag="fb")
            nc.vector.tensor_copy(fa[:], Oa[:])
            nc.scalar.copy(fb[:], Ob[:])
            nc.sync.dma_start(out[0:128, sl(s)], fa[:])
            nc.sync.dma_start(out[128:C, sl(s)], fb[:])
    st.close()


def build_module():
    nc = bacc.Bacc("TRN2")
    io = {}
    io["kv"] = nc.dram_tensor("kv", [C, HWTOT], F32, kind="ExternalInput").ap()
    io["q"] = nc.dram_tensor("q", [C, HWTOT], F32, kind="ExternalInput").ap()
    io["w1t"] = nc.dram_tensor("w1t", [C, C2], F16, kind="ExternalInput").ap()
    io["w2d"] = nc.dram_tensor("w2d", [27, 128, 128], F16,
                               kind="ExternalInput").ap()
    io["w2col"] = nc.dram_tensor("w2col", [128, 27], F32,
                                 kind="ExternalInput").ap()
    io["w2dn"] = nc.dram_tensor("w2dn", [18, 128, 128], F16,
                                kind="ExternalInput").ap()
    io["wpt"] = nc.dram_tensor("wpt", [C, C], F16, kind="ExternalInput").ap()
    io["ident"] = nc.dram_tensor("ident", [128, 128], F16,
                                 kind="ExternalInput").ap()
    io["mask01"] = nc.dram_tensor("mask01", [C, C], F16,
                                  kind="ExternalInput").ap()
    io["scale192"] = nc.dram_tensor("scale192", [C, 1], F32,
                                    kind="ExternalInput").ap()
    io["out"] = nc.dram_tensor("out", [C, HWTOT], F32,
                               kind="ExternalOutput").ap()
    with tile.TileContext(nc) as tc:
        emit_kernel(tc, io)
    nc.compile()
    return nc


def prep_weights(qkv1_w, qkv2_w, proj_w, scale):
    w1 = np.asarray(qkv1_w).reshape(C2, C)
    w1t = np.ascontiguousarray(w1.T).astype(np.float16)
    w2 = np.asarray(qkv2_w).reshape(C2, 9)
    w2d = np.zeros((27, 128, 128), np.float16)
    for mc in range(3):
        for wi in range(9):
            np.fill_diagonal(w2d[mc * 9 + wi], w2[mc * 128:(mc + 1) * 128, wi])
    w2dn = np.zeros((18, 128, 128), np.float16)
    for mc in range(3):
        for ki, (dr, dc) in enumerate(
                [(dr, dc) for dr in (-1, 0, 1) for dc in (-1, 1)]):
            wi = (dr + 1) * 3 + (dc + 1)
            np.fill_diagonal(w2dn[mc * 6 + ki],
                             -w2[mc * 128:(mc + 1) * 128, wi])
    w2col = np.zeros((128, 27), np.float32)
    for mc in range(3):
        w2col[:, mc * 9:(mc + 1) * 9] = w2[mc * 128:(mc + 1) * 128, :]
    wp = np.asarray(proj_w).reshape(C, C)
    wpt = np.ascontiguousarray(wp.T).astype(np.float16)
    ident = np.eye(128, dtype=np.float16)
    mask01 = np.zeros((C, C), np.float16)
    for h in range(HEADS):
        mask01[h * CD:(h + 1) * CD, h * CD:(h + 1) * CD] = 1.0
    scale192 = np.repeat(np.asarray(scale).reshape(HEADS), CD).astype(
        np.float32).reshape(C, 1)
    return {"w1t": w1t, "w2d": w2d, "w2dn": w2dn, "w2col": w2col,
            "wpt": wpt, "ident": ident, "mask01": mask01,
            "scale192": scale192}


_CACHED = {}


def kernel(kv, q, qkv1_w, qkv2_w, proj_w, scale):
    kv = np.asarray(kv, np.float32)
    q = np.asarray(q, np.float32)
    b = kv.shape[0]
    assert b == 8 and kv.shape[1] == C
    wts = prep_weights(qkv1_w, qkv2_w, proj_w, scale)
    if "nc" not in _CACHED:
        nc = build_module()
        nc.m = get_hw_module(nc.m)
        _CACHED["nc"] = nc
    nc = _CACHED["nc"]
    in_maps = []
    for i in range(b):
        m = {"kv": np.ascontiguousarray(kv[i].reshape(C, HWTOT)),
             "q": np.ascontiguousarray(q[i].reshape(C, HWTOT))}
        m.update(wts)
        in_maps.append(m)
    res = run_bass_kernel_spmd(nc, in_maps, core_ids=list(range(8)))
    out = np.stack([res.results[i]["out"].reshape(C, H, W) for i in range(b)])
    return out.astype(np.float32)
